# revision 13
# baseline (speedup 1.0000x reference)
"""BertSelfAttention on 8 TRN2 NeuronCores (Bass/Tile).

Sharding: tensor-parallel over heads. Core c computes heads 2c, 2c+1
(output dims 128c : 128c+128). Each core receives the full hidden states
(pre-transposed and cast to bf16 on the host) plus its slice of the
Q/K/V projection weights, and produces its [B, S, 128] slice of the
context output; the host concatenates slices along the feature axis.

Per-core pipeline (B=4, S=2048, H=1024, NH=16, HD=64; 2 heads/core):
  1. proj:  QT,KT [128, S] (head dim on partitions) and V [S, 128]
            (seq on partitions) via PE matmuls over 8 contraction chunks.
  2. attn:  for each 512-wide query chunk:
              for each 128-wide key block:
                ST[k,q] = KT^T @ QT  (two heads packed into PE row halves,
                                      contraction = head_dim = 64)
                P = exp(ST/8 + mask) on ScalarE (psum -> sbuf bf16)
                OT[d,q] += [V | 1]^T-style matmul: lhsT=[V,ones] (M=65)
                           accumulating both context and row-sums l.
              normalize: transpose OT -> [q, d], multiply by 1/l.
The bv bias is folded in on the host (rows of softmax sum to one, so
ctx(V + bv) = ctx(V) + bv exactly).
"""

import numpy as np
import ml_dtypes

import concourse.bass as bass
import concourse.mybir as mybir
import concourse.tile as tile
from concourse import bass_utils
from concourse.masks import make_identity
from concourse.vector_clock import ScopedClock

B, S, H, NH, HD = 4, 2048, 1024, 16, 64
N_CORES = 8
DH = H // N_CORES          # 128 output dims per core (2 heads)
P = 128
QC = 512                   # query chunk (psum bank width in fp32)
NQC = S // QC              # 4
NKB = S // P               # 16 key blocks
NHC = H // P               # 8 contraction chunks for the projections
BF16 = mybir.dt.bfloat16
F32 = mybir.dt.float32


_COMPUTE_INSTS = (
    "InstNoOp",
    "InstActivation",
    "InstMatmult",
    "InstLdweights",
    "InstTensorTensor",
    "InstTensorCopy",
    "InstTensorScalarPtr",
    "InstTensorScalar",
    "InstReciprocal",
    "InstMemset",
    "InstTensorReduce",
)
_DROP_SELF_WAITS = False
_ENGINE_SEM_PREFIX = {
    mybir.EngineType.PE: "PE_",
    mybir.EngineType.Activation: "Activation_",
    mybir.EngineType.DVE: "DVE_",
    mybir.EngineType.Pool: "Pool_",
}


def _split_multi_waits(nc):
    # walrus in this container accepts at most ONE sync wait per
    # instruction; hoist extra waits onto preceding same-engine NOPs.
    # Along the way, drop same-engine self-waits on compute instructions:
    # engines execute their own stream in order, so a wait on the engine's
    # own completion semaphore only forces the sequencer to stall until the
    # engine drains (a throughput bubble), without adding any ordering.
    n = 0
    for bb in nc.m.functions[0].blocks:
        new_insts = []
        for inst in bb.instructions:
            si = inst.sync_info
            if si is not None and si.on_wait:
                waits = list(si.on_wait)
                pref = _ENGINE_SEM_PREFIX.get(inst.engine) if _DROP_SELF_WAITS else None
                if pref is not None and type(inst).__name__ in _COMPUTE_INSTS:
                    kept = [
                        w
                        for w in waits
                        if not (w.ant_name or "").startswith(pref)
                    ]
                    waits = kept
                if not waits and type(inst).__name__ == "InstNoOp" and not (
                    si.on_update
                ):
                    continue  # noop with nothing left to do
                for w in waits[:-1]:
                    n += 1
                    new_insts.append(
                        mybir.InstNoOp(
                            name=f"waitsplit_{n}",
                            engine=inst.engine,
                            bass_nofuse=True,
                            sync_info=mybir.SyncInfo(on_wait=[w], on_update=[]),
                        )
                    )
                si.on_wait = waits[-1:]
            new_insts.append(inst)
        bb.instructions[:] = new_insts


def build_bass(reps=1):
    nc = bass.Bass("TRN2", target_bir_lowering=False, debug=False)
    xt = nc.dram_tensor("xt", [B, H, S], BF16, kind="ExternalInput").ap()
    wqt = nc.dram_tensor("wqt", [H, DH], BF16, kind="ExternalInput").ap()
    wkt = nc.dram_tensor("wkt", [H, DH], BF16, kind="ExternalInput").ap()
    wvt = nc.dram_tensor("wvt", [H, DH], BF16, kind="ExternalInput").ap()
    bqv = nc.dram_tensor("bqv", [DH], F32, kind="ExternalInput").ap()
    bkv = nc.dram_tensor("bkv", [DH], F32, kind="ExternalInput").ap()
    mask = nc.dram_tensor("mask", [B, S], F32, kind="ExternalInput").ap()
    out = nc.dram_tensor("out", [B, S, DH], F32, kind="ExternalOutput").ap()

    with tile.TileContext(nc) as tc:
        from contextlib import ExitStack

        with ExitStack() as ctx:
            consts = ctx.enter_context(tc.tile_pool(name="consts", bufs=1))
            xt_pool = ctx.enter_context(tc.tile_pool(name="xt", bufs=2))
            qkt_pool = ctx.enter_context(tc.tile_pool(name="qkt", bufs=2))
            von_pool = ctx.enter_context(tc.tile_pool(name="von", bufs=2))
            ex_pool = ctx.enter_context(tc.tile_pool(name="ex", bufs=4))
            s01_pool = ctx.enter_context(tc.tile_pool(name="s01", bufs=2))
            rb_pool = ctx.enter_context(tc.tile_pool(name="rb", bufs=4))
            osb_pool = ctx.enter_context(tc.tile_pool(name="osb", bufs=2))
            mask_pool = ctx.enter_context(tc.tile_pool(name="maskp", bufs=2))
            ps_misc = ctx.enter_context(tc.tile_pool(name="ps_misc", bufs=2, space="PSUM"))
            ps_st = ctx.enter_context(tc.tile_pool(name="ps_st", bufs=2, space="PSUM"))
            ps_ot = ctx.enter_context(tc.tile_pool(name="ps_ot", bufs=2, space="PSUM"))

            # constants
            wq_sb = consts.tile([P, NHC, DH], BF16, name="wq_sb")
            wk_sb = consts.tile([P, NHC, DH], BF16, name="wk_sb")
            wv_sb = consts.tile([P, NHC, DH], BF16, name="wv_sb")
            nc.sync.dma_start(wq_sb[:], wqt.rearrange("(hc p) d -> p hc d", p=P))
            nc.sync.dma_start(wk_sb[:], wkt.rearrange("(hc p) d -> p hc d", p=P))
            nc.sync.dma_start(wv_sb[:], wvt.rearrange("(hc p) d -> p hc d", p=P))
            bq_sb = consts.tile([P, 1], F32, name="bq_sb")
            bk_sb = consts.tile([P, 1], F32, name="bk_sb")
            nc.sync.dma_start(bq_sb[:], bqv[:, None])
            nc.sync.dma_start(bk_sb[:], bkv[:, None])
            ident = consts.tile([P, P], F32, name="ident")
            make_identity(nc, ident[:])

            def start_b(b):
                """Allocate per-batch tiles, issue input DMAs, and build the
                list of projection work units (each ~8 matmuls + 1 copy)."""
                st = {}
                st["xt"] = xt_pool.tile([P, NHC, S], BF16, name="xt_b", tag="xt_b")
                xr = xt[b].rearrange("(hc p) s -> p hc s", p=P)
                for hc in range(NHC):
                    # per-chunk DMAs so the first projection matmuls can
                    # start before the whole 4MB slice has landed
                    nc.sync.dma_start(st["xt"][:, hc, :], xr[:, hc, :])
                st["mask"] = mask_pool.tile([P, NKB], F32, name="mask_b", tag="mask_b")
                nc.sync.dma_start(
                    st["mask"][:], mask[b].rearrange("(kb p) -> p kb", p=P)
                )
                st["qt"] = qkt_pool.tile([P, S], BF16, name="qt", tag="qt")
                st["kt"] = qkt_pool.tile([P, S], BF16, name="kt", tag="kt")
                st["von"] = von_pool.tile(
                    [P, NKB, 2 * (HD + 1)], BF16, name="von", tag="von"
                )
                nc.vector.memset(st["von"][:, :, HD:HD + 1], 1.0)
                nc.vector.memset(st["von"][:, :, 2 * HD + 1:2 * HD + 2], 1.0)
                # Unit order matters: attention on (b, qc=0) needs all kt
                # chunks, qt chunk 0, and the first few von blocks. pq/pk
                # units are 256-wide halves so injected bursts stay short.
                st["units"] = (
                    [("pk", i) for i in range(2 * NQC)]
                    + [("pq", 0), ("pq", 1)]
                    + [("pv", kb) for kb in range(4)]
                    + [("pq", 2), ("pq", 3), ("pv", 4), ("pq", 4), ("pq", 5),
                       ("pv", 5), ("pq", 6), ("pq", 7)]
                    + [("pv", kb) for kb in range(6, NKB)]
                )
                return st

            HQ = QC // 2

            def emit_unit(st, unit):
                kind, idx = unit
                if kind in ("pq", "pk"):
                    w_sb = wq_sb if kind == "pq" else wk_sb
                    b_sb = bq_sb if kind == "pq" else bk_sb
                    dest = st["qt"] if kind == "pq" else st["kt"]
                    pp = ps_misc.tile([P, HQ], F32, name=kind, tag="misc")
                    for h in range(NHC):
                        nc.tensor.matmul(
                            pp[:],
                            lhsT=w_sb[:, h, :],
                            rhs=st["xt"][:, h, idx * HQ:(idx + 1) * HQ],
                            start=(h == 0),
                            stop=(h == NHC - 1),
                        )
                    nc.vector.tensor_tensor(
                        dest[:, idx * HQ:(idx + 1) * HQ],
                        pp[:],
                        b_sb[:].to_broadcast((P, HQ)),
                        mybir.AluOpType.add,
                    )
                else:  # pv: V block idx in [s, d] layout
                    pv = ps_misc.tile([P, P], F32, name="pv", tag="misc")
                    for h in range(NHC):
                        nc.tensor.matmul(
                            pv[:],
                            lhsT=st["xt"][:, h, idx * P:(idx + 1) * P],
                            rhs=wv_sb[:, h, :],
                            start=(h == 0),
                            stop=(h == NHC - 1),
                        )
                    nc.vector.tensor_copy(st["von"][:, idx, 0:HD], pv[:, 0:HD])
                    nc.vector.tensor_copy(
                        st["von"][:, idx, HD + 1:2 * HD + 1], pv[:, HD:2 * HD]
                    )

            seq = [b for _ in range(reps) for b in range(B)]
            state = {}
            # prologue for the first batch: enough projections to start
            # attention (all kt chunks, qt chunk 0, first 4 V blocks);
            # the rest is injected into the first attention qc's k-loop.
            state[0] = start_b(seq[0])
            for u in state[0]["units"][:14]:
                emit_unit(state[0], u)
            own_pending = list(state[0]["units"][14:])

            for pos, b in enumerate(seq):
                stt = state[pos]
                mask_b = stt["mask"]
                qt = stt["qt"]
                kt = stt["kt"]
                von = stt["von"]
                if pos + 1 < len(seq):
                    state[pos + 1] = start_b(seq[pos + 1])
                    next_units = list(state[pos + 1]["units"])
                else:
                    next_units = []
                state.pop(pos - 1, None)

                # ---- attention (with projection work injected) ----
                inj_i = 0
                for qc in range(NQC):
                    qsl = slice(qc * QC, (qc + 1) * QC)
                    ot0 = ps_ot.tile([P, QC], F32, name="ot0", tag="ot")
                    ot1 = ps_ot.tile([P, QC], F32, name="ot1", tag="ot")
                    for kb in range(NKB):
                        if own_pending:
                            for u in own_pending[:2]:
                                emit_unit(stt, u)
                            del own_pending[:2]
                        elif next_units and inj_i < len(next_units):
                            it = qc * NKB + kb
                            target = min(
                                len(next_units),
                                it * len(next_units) // (NQC * NKB - 16) + 1,
                            )
                            while inj_i < target:
                                emit_unit(state[pos + 1], next_units[inj_i])
                                inj_i += 1
                        stp = ps_st.tile([P, 2 * QC], F32, name="stp")
                        nc.tensor.matmul(
                            stp[:, 0:QC],
                            lhsT=kt[0:HD, kb * P:(kb + 1) * P],
                            rhs=qt[0:HD, qsl],
                            start=True,
                            stop=True,
                        )
                        nc.tensor.matmul(
                            stp[:, QC:2 * QC],
                            lhsT=kt[HD:2 * HD, kb * P:(kb + 1) * P],
                            rhs=qt[HD:2 * HD, qsl],
                            start=True,
                            stop=True,
                        )
                        ex = ex_pool.tile([P, 2 * QC], BF16, name="ex")
                        nc.scalar.activation(
                            ex[:],
                            stp[:],
                            mybir.ActivationFunctionType.Exp,
                            bias=mask_b[:, kb:kb + 1],
                            scale=1.0 / np.sqrt(HD),
                        )
                        nc.tensor.matmul(
                            ot0[0:HD + 1, :],
                            lhsT=von[:, kb, 0:HD + 1],
                            rhs=ex[:, 0:QC],
                            start=(kb == 0),
                            stop=(kb == NKB - 1),
                        )
                        nc.tensor.matmul(
                            ot1[0:HD + 1, :],
                            lhsT=von[:, kb, HD + 1:2 * HD + 2],
                            rhs=ex[:, QC:2 * QC],
                            start=(kb == 0),
                            stop=(kb == NKB - 1),
                        )

                    # normalize + transpose to [q, d] and store
                    s0 = s01_pool.tile([HD + 1, QC], F32, name="s0", tag="s01")
                    s1 = s01_pool.tile([HD + 1, QC], F32, name="s1", tag="s01")
                    nc.vector.tensor_copy(s0[:], ot0[0:HD + 1, :])
                    nc.vector.tensor_copy(s1[:], ot1[0:HD + 1, :])
                    osb = osb_pool.tile([P, QC // P, DH], F32, name="osb")
                    for j in range(QC // P):
                        jsl = slice(j * P, (j + 1) * P)
                        o2t0 = ps_misc.tile([P, HD + 1], F32, name="o2t0", tag="misc")
                        nc.tensor.transpose(
                            o2t0[:], s0[:, jsl], ident[0:HD + 1, 0:HD + 1]
                        )
                        o2t1 = ps_misc.tile([P, HD + 1], F32, name="o2t1", tag="misc")
                        nc.tensor.transpose(
                            o2t1[:], s1[:, jsl], ident[0:HD + 1, 0:HD + 1]
                        )
                        rb0 = rb_pool.tile([P, 1], F32, name="rb0", tag="rb")
                        rb1 = rb_pool.tile([P, 1], F32, name="rb1", tag="rb")
                        nc.vector.reciprocal(rb0[:], o2t0[:, HD:HD + 1])
                        nc.vector.reciprocal(rb1[:], o2t1[:, HD:HD + 1])
                        nc.vector.tensor_scalar_mul(osb[:, j, 0:HD], o2t0[:, 0:HD], rb0[:])
                        nc.vector.tensor_scalar_mul(osb[:, j, HD:2 * HD], o2t1[:, 0:HD], rb1[:])
                    nc.sync.dma_start(
                        out[b].rearrange("(a p) d -> p a d", p=P)[
                            :, qc * (QC // P):(qc + 1) * (QC // P), :
                        ],
                        osb[:],
                    )
    _split_multi_waits(nc)
    return nc


def host_prep(hidden_states, attention_mask, Wq, bq, Wk, bk, Wv, bv):
    xt_np = np.ascontiguousarray(
        np.asarray(hidden_states).transpose(0, 2, 1)
    ).astype(ml_dtypes.bfloat16)
    mask_np = np.ascontiguousarray(
        np.asarray(attention_mask).reshape(B, S)
    ).astype(np.float32)
    in_maps = []
    for c in range(N_CORES):
        dsl = slice(c * DH, (c + 1) * DH)
        in_maps.append(
            {
                "xt": xt_np,
                "wqt": np.ascontiguousarray(np.asarray(Wq)[dsl, :].T).astype(ml_dtypes.bfloat16),
                "wkt": np.ascontiguousarray(np.asarray(Wk)[dsl, :].T).astype(ml_dtypes.bfloat16),
                "wvt": np.ascontiguousarray(np.asarray(Wv)[dsl, :].T).astype(ml_dtypes.bfloat16),
                "bqv": np.ascontiguousarray(np.asarray(bq)[dsl]).astype(np.float32),
                "bkv": np.ascontiguousarray(np.asarray(bk)[dsl]).astype(np.float32),
                "mask": mask_np,
            }
        )
    return in_maps


def gather(results, bv):
    out = np.empty((B, S, H), np.float32)
    for c in range(N_CORES):
        out[:, :, c * DH:(c + 1) * DH] = results[c]["out"]
    # bv folded on the host: softmax rows sum to 1, so ctx(V+bv)=ctx(V)+bv
    out += np.asarray(bv).astype(np.float32)[None, None, :]
    return out


def make_runner(nc, in_maps):
    """Build a reusable jitted 8-core runner for `nc` (mirrors
    bass2jax.run_bass_via_pjrt's multi-core path, but keeps the jitted
    callable so repeated executions don't re-lower)."""
    import jax
    from jax.sharding import Mesh, NamedSharding, PartitionSpec
    from jax.experimental.shard_map import shard_map
    from concourse import bass2jax

    bass2jax.install_neuronx_cc_hook()
    partition_name = nc.partition_id_tensor.name if nc.partition_id_tensor else None
    in_names, out_names, out_avals, zero_outs = [], [], [], []
    for alloc in nc.m.functions[0].allocations:
        if not isinstance(alloc, mybir.MemoryLocationSet):
            continue
        name = alloc.memorylocations[0].name
        if alloc.kind == "ExternalInput":
            if name != partition_name:
                in_names.append(name)
        elif alloc.kind == "ExternalOutput":
            out_names.append(name)
            shape = tuple(alloc.tensor_shape)
            dtype = mybir.dt.np(alloc.dtype)
            out_avals.append(jax.core.ShapedArray(shape, dtype))
            zero_outs.append(np.zeros(shape, dtype))
    n_params = len(in_names)
    n_outs = len(out_avals)
    all_in = list(in_names) + list(out_names)
    if partition_name is not None:
        all_in.append(partition_name)

    def _body(*args):
        operands = list(args)
        if partition_name is not None:
            operands.append(bass2jax.partition_id_tensor())
        outs = bass2jax._bass_exec_p.bind(
            *operands,
            out_avals=tuple(out_avals),
            in_names=tuple(all_in),
            out_names=tuple(out_names),
            lowering_input_output_aliases=(),
            sim_require_finite=True,
            sim_require_nnan=True,
            nc=nc,
        )
        return tuple(outs)

    devices = jax.devices()[:N_CORES]
    mesh = Mesh(np.asarray(devices), ("core",))
    sharded = jax.jit(
        shard_map(
            _body,
            mesh=mesh,
            in_specs=(PartitionSpec("core"),) * (n_params + n_outs),
            out_specs=(PartitionSpec("core"),) * n_outs,
            check_rep=False,
        ),
        keep_unused=True,
    )
    per_core = [[np.asarray(m[name]) for name in in_names[:n_params]] for m in in_maps]
    concat_in = [
        np.concatenate([per_core[c][i] for c in range(N_CORES)], axis=0)
        for i in range(n_params)
    ]
    concat_zeros = [
        np.zeros((N_CORES * z.shape[0], *z.shape[1:]), z.dtype) for z in zero_outs
    ]
    sh = NamedSharding(mesh, PartitionSpec("core"))
    args_dev = [jax.device_put(a, sh) for a in concat_in] + [
        jax.device_put(a, sh) for a in concat_zeros
    ]

    def run():
        import jax as _jax

        outs = sharded(*args_dev)
        _jax.block_until_ready(outs)
        return [
            {
                name: np.asarray(outs[i]).reshape(N_CORES, *out_avals[i].shape)[c]
                for i, name in enumerate(out_names)
            }
            for c in range(N_CORES)
        ]

    return run


def kernel(hidden_states, attention_mask, Wq, bq, Wk, bk, Wv, bv):
    in_maps = host_prep(hidden_states, attention_mask, Wq, bq, Wk, bk, Wv, bv)
    nc = build_bass()
    res = bass_utils.run_bass_kernel_spmd(nc, in_maps, core_ids=list(range(N_CORES)))
    return gather(res.results, bv)


# revision 14
# speedup vs baseline: 1.0193x; 1.0193x over previous
"""BertSelfAttention on 8 TRN2 NeuronCores (Bass/Tile).

Sharding: tensor-parallel over heads. Core c computes heads 2c, 2c+1
(output dims 128c : 128c+128). Each core receives the full hidden states
(pre-transposed and cast to bf16 on the host) plus its slice of the
Q/K/V projection weights, and produces its [B, S, 128] slice of the
context output; the host concatenates slices along the feature axis.

Per-core pipeline (B=4, S=2048, H=1024, NH=16, HD=64; 2 heads/core):
  1. proj:  QT,KT [128, S] (head dim on partitions) and V [S, 128]
            (seq on partitions) via PE matmuls over 8 contraction chunks.
  2. attn:  for each 512-wide query chunk:
              for each 128-wide key block:
                ST[k,q] = KT^T @ QT  (two heads packed into PE row halves,
                                      contraction = head_dim = 64)
                P = exp(ST/8 + mask) on ScalarE (psum -> sbuf bf16)
                OT[d,q] += [V | 1]^T-style matmul: lhsT=[V,ones] (M=65)
                           accumulating both context and row-sums l.
              normalize: transpose OT -> [q, d], multiply by 1/l.
The bv bias is folded in on the host (rows of softmax sum to one, so
ctx(V + bv) = ctx(V) + bv exactly).
"""

import numpy as np
import ml_dtypes

import concourse.bass as bass
import concourse.mybir as mybir
import concourse.tile as tile
from concourse import bass_utils
from concourse.masks import make_identity
from concourse.vector_clock import ScopedClock

B, S, H, NH, HD = 4, 2048, 1024, 16, 64
N_CORES = 8
DH = H // N_CORES          # 128 output dims per core (2 heads)
P = 128
QC = 512                   # query chunk (psum bank width in fp32)
NQC = S // QC              # 4
NKB = S // P               # 16 key blocks
NHC = H // P               # 8 contraction chunks for the projections
BF16 = mybir.dt.bfloat16
F32 = mybir.dt.float32


_COMPUTE_INSTS = (
    "InstNoOp",
    "InstActivation",
    "InstMatmult",
    "InstLdweights",
    "InstTensorTensor",
    "InstTensorCopy",
    "InstTensorScalarPtr",
    "InstTensorScalar",
    "InstReciprocal",
    "InstMemset",
    "InstTensorReduce",
)
_DROP_SELF_WAITS = False
_ENGINE_SEM_PREFIX = {
    mybir.EngineType.PE: "PE_",
    mybir.EngineType.Activation: "Activation_",
    mybir.EngineType.DVE: "DVE_",
    mybir.EngineType.Pool: "Pool_",
}


def _split_multi_waits(nc):
    # walrus in this container accepts at most ONE sync wait per
    # instruction; hoist extra waits onto preceding same-engine NOPs.
    # Along the way, drop same-engine self-waits on compute instructions:
    # engines execute their own stream in order, so a wait on the engine's
    # own completion semaphore only forces the sequencer to stall until the
    # engine drains (a throughput bubble), without adding any ordering.
    n = 0
    for bb in nc.m.functions[0].blocks:
        new_insts = []
        for inst in bb.instructions:
            si = inst.sync_info
            if si is not None and si.on_wait:
                waits = list(si.on_wait)
                pref = _ENGINE_SEM_PREFIX.get(inst.engine) if _DROP_SELF_WAITS else None
                if pref is not None and type(inst).__name__ in _COMPUTE_INSTS:
                    kept = [
                        w
                        for w in waits
                        if not (w.ant_name or "").startswith(pref)
                    ]
                    waits = kept
                if not waits and type(inst).__name__ == "InstNoOp" and not (
                    si.on_update
                ):
                    continue  # noop with nothing left to do
                for w in waits[:-1]:
                    n += 1
                    new_insts.append(
                        mybir.InstNoOp(
                            name=f"waitsplit_{n}",
                            engine=inst.engine,
                            bass_nofuse=True,
                            sync_info=mybir.SyncInfo(on_wait=[w], on_update=[]),
                        )
                    )
                si.on_wait = waits[-1:]
            new_insts.append(inst)
        bb.instructions[:] = new_insts


def build_bass(reps=1):
    nc = bass.Bass("TRN2", target_bir_lowering=False, debug=False)
    xt = nc.dram_tensor("xt", [B, H, S], BF16, kind="ExternalInput").ap()
    wqt = nc.dram_tensor("wqt", [H, DH], BF16, kind="ExternalInput").ap()
    wkt = nc.dram_tensor("wkt", [H, DH], BF16, kind="ExternalInput").ap()
    wvt = nc.dram_tensor("wvt", [H, DH], BF16, kind="ExternalInput").ap()
    bqv = nc.dram_tensor("bqv", [DH], F32, kind="ExternalInput").ap()
    bkv = nc.dram_tensor("bkv", [DH], F32, kind="ExternalInput").ap()
    mask = nc.dram_tensor("mask", [B, S], F32, kind="ExternalInput").ap()
    out = nc.dram_tensor("out", [B, S, DH], F32, kind="ExternalOutput").ap()

    with tile.TileContext(nc) as tc:
        from contextlib import ExitStack

        with ExitStack() as ctx:
            consts = ctx.enter_context(tc.tile_pool(name="consts", bufs=1))
            xt_pool = ctx.enter_context(tc.tile_pool(name="xt", bufs=2))
            qkt_pool = ctx.enter_context(tc.tile_pool(name="qkt", bufs=2))
            von_pool = ctx.enter_context(tc.tile_pool(name="von", bufs=2))
            ex_pool = ctx.enter_context(tc.tile_pool(name="ex", bufs=4))
            s01_pool = ctx.enter_context(tc.tile_pool(name="s01", bufs=2))
            rb_pool = ctx.enter_context(tc.tile_pool(name="rb", bufs=4))
            osb_pool = ctx.enter_context(tc.tile_pool(name="osb", bufs=2))
            mask_pool = ctx.enter_context(tc.tile_pool(name="maskp", bufs=2))
            ps_misc = ctx.enter_context(tc.tile_pool(name="ps_misc", bufs=2, space="PSUM"))
            ps_st = ctx.enter_context(tc.tile_pool(name="ps_st", bufs=2, space="PSUM"))
            ps_ot = ctx.enter_context(tc.tile_pool(name="ps_ot", bufs=2, space="PSUM"))

            # constants
            wq_sb = consts.tile([P, NHC, DH], BF16, name="wq_sb")
            wk_sb = consts.tile([P, NHC, DH], BF16, name="wk_sb")
            wv_sb = consts.tile([P, NHC, DH], BF16, name="wv_sb")
            nc.sync.dma_start(wq_sb[:], wqt.rearrange("(hc p) d -> p hc d", p=P))
            nc.sync.dma_start(wk_sb[:], wkt.rearrange("(hc p) d -> p hc d", p=P))
            nc.sync.dma_start(wv_sb[:], wvt.rearrange("(hc p) d -> p hc d", p=P))
            bq_sb = consts.tile([P, 1], F32, name="bq_sb")
            bk_sb = consts.tile([P, 1], F32, name="bk_sb")
            nc.sync.dma_start(bq_sb[:], bqv[:, None])
            nc.sync.dma_start(bk_sb[:], bkv[:, None])
            ident = consts.tile([P, P], F32, name="ident")
            make_identity(nc, ident[:])

            def start_b(b):
                """Allocate per-batch tiles, issue input DMAs, and build the
                list of projection work units (each ~8 matmuls + 1 copy)."""
                st = {}
                st["xt"] = xt_pool.tile([P, NHC, S], BF16, name="xt_b", tag="xt_b")
                xr = xt[b].rearrange("(hc p) s -> p hc s", p=P)
                for hc in range(NHC):
                    # per-chunk DMAs so the first projection matmuls can
                    # start before the whole 4MB slice has landed
                    nc.sync.dma_start(st["xt"][:, hc, :], xr[:, hc, :])
                st["mask"] = mask_pool.tile([P, NKB], F32, name="mask_b", tag="mask_b")
                nc.sync.dma_start(
                    st["mask"][:], mask[b].rearrange("(kb p) -> p kb", p=P)
                )
                st["qt"] = qkt_pool.tile([P, S], BF16, name="qt", tag="qt")
                st["kt"] = qkt_pool.tile([P, S], BF16, name="kt", tag="kt")
                st["von"] = von_pool.tile(
                    [P, NKB, 2 * (HD + 1)], BF16, name="von", tag="von"
                )
                nc.vector.memset(st["von"][:, :, HD:HD + 1], 1.0)
                nc.vector.memset(st["von"][:, :, 2 * HD + 1:2 * HD + 2], 1.0)
                # Unit order matters: attention on (b, qc=0) needs all kt
                # chunks, qt chunk 0, and the first few von blocks. pq/pk
                # units are 256-wide halves so injected bursts stay short.
                st["units"] = (
                    [("pk", i) for i in range(2 * NQC)]
                    + [("pq", 0), ("pq", 1)]
                    + [("pv", kb) for kb in range(4)]
                    + [("pq", 2), ("pq", 3), ("pv", 4), ("pq", 4), ("pq", 5),
                       ("pv", 5), ("pq", 6), ("pq", 7)]
                    + [("pv", kb) for kb in range(6, NKB)]
                )
                return st

            HQ = QC // 2

            def emit_unit(st, unit):
                kind, idx = unit
                if kind in ("pq", "pk"):
                    w_sb = wq_sb if kind == "pq" else wk_sb
                    b_sb = bq_sb if kind == "pq" else bk_sb
                    dest = st["qt"] if kind == "pq" else st["kt"]
                    pp = ps_misc.tile([P, HQ], F32, name=kind, tag="misc")
                    for h in range(NHC):
                        nc.tensor.matmul(
                            pp[:],
                            lhsT=w_sb[:, h, :],
                            rhs=st["xt"][:, h, idx * HQ:(idx + 1) * HQ],
                            start=(h == 0),
                            stop=(h == NHC - 1),
                        )
                    nc.vector.tensor_tensor(
                        dest[:, idx * HQ:(idx + 1) * HQ],
                        pp[:],
                        b_sb[:].to_broadcast((P, HQ)),
                        mybir.AluOpType.add,
                    )
                else:  # pv: V block idx in [s, d] layout
                    pv = ps_misc.tile([P, P], F32, name="pv", tag="misc")
                    for h in range(NHC):
                        nc.tensor.matmul(
                            pv[:],
                            lhsT=st["xt"][:, h, idx * P:(idx + 1) * P],
                            rhs=wv_sb[:, h, :],
                            start=(h == 0),
                            stop=(h == NHC - 1),
                        )
                    nc.vector.tensor_copy(st["von"][:, idx, 0:HD], pv[:, 0:HD])
                    nc.vector.tensor_copy(
                        st["von"][:, idx, HD + 1:2 * HD + 1], pv[:, HD:2 * HD]
                    )

            seq = [b for _ in range(reps) for b in range(B)]
            state = {}
            # prologue for the first batch: enough projections to start
            # attention (all kt chunks, qt chunk 0, first 4 V blocks);
            # the rest is injected into the first attention qc's k-loop.
            state[0] = start_b(seq[0])
            for u in state[0]["units"][:14]:
                emit_unit(state[0], u)
            own_pending = list(state[0]["units"][14:])

            for pos, b in enumerate(seq):
                stt = state[pos]
                mask_b = stt["mask"]
                qt = stt["qt"]
                kt = stt["kt"]
                von = stt["von"]
                if pos + 1 < len(seq):
                    state[pos + 1] = start_b(seq[pos + 1])
                    next_units = list(state[pos + 1]["units"])
                else:
                    next_units = []
                state.pop(pos - 1, None)

                # ---- attention (with projection work injected) ----
                inj_i = 0
                for qc in range(NQC):
                    qsl = slice(qc * QC, (qc + 1) * QC)
                    ot0 = ps_ot.tile([P, QC], F32, name="ot0", tag="ot")
                    ot1 = ps_ot.tile([P, QC], F32, name="ot1", tag="ot")
                    for kb in range(NKB):
                        if own_pending:
                            for u in own_pending[:2]:
                                emit_unit(stt, u)
                            del own_pending[:2]
                        elif next_units and inj_i < len(next_units):
                            it = qc * NKB + kb
                            target = min(
                                len(next_units),
                                it * len(next_units) // (NQC * NKB - 16) + 1,
                            )
                            while inj_i < target:
                                emit_unit(state[pos + 1], next_units[inj_i])
                                inj_i += 1
                        stp = ps_st.tile([P, 2 * QC], F32, name="stp")
                        nc.tensor.matmul(
                            stp[:, 0:QC],
                            lhsT=kt[0:HD, kb * P:(kb + 1) * P],
                            rhs=qt[0:HD, qsl],
                            start=True,
                            stop=True,
                        )
                        nc.tensor.matmul(
                            stp[:, QC:2 * QC],
                            lhsT=kt[HD:2 * HD, kb * P:(kb + 1) * P],
                            rhs=qt[HD:2 * HD, qsl],
                            start=True,
                            stop=True,
                        )
                        ex = ex_pool.tile([P, 2 * QC], BF16, name="ex")
                        nc.scalar.activation(
                            ex[:],
                            stp[:],
                            mybir.ActivationFunctionType.Exp,
                            bias=mask_b[:, kb:kb + 1],
                            scale=1.0 / np.sqrt(HD),
                        )
                        nc.tensor.matmul(
                            ot0[0:HD + 1, :],
                            lhsT=von[:, kb, 0:HD + 1],
                            rhs=ex[:, 0:QC],
                            start=(kb == 0),
                            stop=(kb == NKB - 1),
                        )
                        nc.tensor.matmul(
                            ot1[0:HD + 1, :],
                            lhsT=von[:, kb, HD + 1:2 * HD + 2],
                            rhs=ex[:, QC:2 * QC],
                            start=(kb == 0),
                            stop=(kb == NKB - 1),
                        )

                    # normalize + transpose to [q, d] and store
                    s0 = s01_pool.tile([HD + 1, QC], F32, name="s0", tag="s01")
                    s1 = s01_pool.tile([HD + 1, QC], F32, name="s1", tag="s01")
                    nc.vector.tensor_copy(s0[:], ot0[0:HD + 1, :])
                    nc.vector.tensor_copy(s1[:], ot1[0:HD + 1, :])
                    osb = osb_pool.tile([P, QC // P, DH], F32, name="osb")
                    for j in range(QC // P):
                        jsl = slice(j * P, (j + 1) * P)
                        o2t0 = ps_misc.tile([P, HD + 1], F32, name="o2t0", tag="misc")
                        nc.tensor.transpose(
                            o2t0[:], s0[:, jsl], ident[0:HD + 1, 0:HD + 1]
                        )
                        o2t1 = ps_misc.tile([P, HD + 1], F32, name="o2t1", tag="misc")
                        nc.tensor.transpose(
                            o2t1[:], s1[:, jsl], ident[0:HD + 1, 0:HD + 1]
                        )
                        rb0 = rb_pool.tile([P, 1], F32, name="rb0", tag="rb")
                        rb1 = rb_pool.tile([P, 1], F32, name="rb1", tag="rb")
                        nc.vector.reciprocal(rb0[:], o2t0[:, HD:HD + 1])
                        nc.vector.reciprocal(rb1[:], o2t1[:, HD:HD + 1])
                        nc.vector.tensor_scalar_mul(osb[:, j, 0:HD], o2t0[:, 0:HD], rb0[:])
                        nc.vector.tensor_scalar_mul(osb[:, j, HD:2 * HD], o2t1[:, 0:HD], rb1[:])
                    nc.sync.dma_start(
                        out[b].rearrange("(a p) d -> p a d", p=P)[
                            :, qc * (QC // P):(qc + 1) * (QC // P), :
                        ],
                        osb[:],
                    )
    _split_multi_waits(nc)
    return nc


def host_prep(hidden_states, attention_mask, Wq, bq, Wk, bk, Wv, bv):
    xt_np = np.ascontiguousarray(
        np.asarray(hidden_states).transpose(0, 2, 1)
    ).astype(ml_dtypes.bfloat16)
    mask_np = np.ascontiguousarray(
        np.asarray(attention_mask).reshape(B, S)
    ).astype(np.float32)
    in_maps = []
    for c in range(N_CORES):
        dsl = slice(c * DH, (c + 1) * DH)
        in_maps.append(
            {
                "xt": xt_np,
                "wqt": np.ascontiguousarray(np.asarray(Wq)[dsl, :].T).astype(ml_dtypes.bfloat16),
                "wkt": np.ascontiguousarray(np.asarray(Wk)[dsl, :].T).astype(ml_dtypes.bfloat16),
                "wvt": np.ascontiguousarray(np.asarray(Wv)[dsl, :].T).astype(ml_dtypes.bfloat16),
                "bqv": np.ascontiguousarray(np.asarray(bq)[dsl]).astype(np.float32),
                "bkv": np.ascontiguousarray(np.asarray(bk)[dsl]).astype(np.float32),
                "mask": mask_np,
            }
        )
    return in_maps


def gather(results, bv):
    out = np.empty((B, S, H), np.float32)
    for c in range(N_CORES):
        out[:, :, c * DH:(c + 1) * DH] = results[c]["out"]
    # bv folded on the host: softmax rows sum to 1, so ctx(V+bv)=ctx(V)+bv
    out += np.asarray(bv).astype(np.float32)[None, None, :]
    return out


def make_runner(nc, in_maps):
    """Build a reusable jitted 8-core runner for `nc` (mirrors
    bass2jax.run_bass_via_pjrt's multi-core path, but keeps the jitted
    callable so repeated executions don't re-lower)."""
    import jax
    from jax.sharding import Mesh, NamedSharding, PartitionSpec
    from jax.experimental.shard_map import shard_map
    from concourse import bass2jax

    bass2jax.install_neuronx_cc_hook()
    partition_name = nc.partition_id_tensor.name if nc.partition_id_tensor else None
    in_names, out_names, out_avals, zero_outs = [], [], [], []
    for alloc in nc.m.functions[0].allocations:
        if not isinstance(alloc, mybir.MemoryLocationSet):
            continue
        name = alloc.memorylocations[0].name
        if alloc.kind == "ExternalInput":
            if name != partition_name:
                in_names.append(name)
        elif alloc.kind == "ExternalOutput":
            out_names.append(name)
            shape = tuple(alloc.tensor_shape)
            dtype = mybir.dt.np(alloc.dtype)
            out_avals.append(jax.core.ShapedArray(shape, dtype))
            zero_outs.append(np.zeros(shape, dtype))
    n_params = len(in_names)
    n_outs = len(out_avals)
    all_in = list(in_names) + list(out_names)
    if partition_name is not None:
        all_in.append(partition_name)

    def _body(*args):
        operands = list(args)
        if partition_name is not None:
            operands.append(bass2jax.partition_id_tensor())
        outs = bass2jax._bass_exec_p.bind(
            *operands,
            out_avals=tuple(out_avals),
            in_names=tuple(all_in),
            out_names=tuple(out_names),
            lowering_input_output_aliases=(),
            sim_require_finite=True,
            sim_require_nnan=True,
            nc=nc,
        )
        return tuple(outs)

    devices = jax.devices()[:N_CORES]
    mesh = Mesh(np.asarray(devices), ("core",))
    sharded = jax.jit(
        shard_map(
            _body,
            mesh=mesh,
            in_specs=(PartitionSpec("core"),) * (n_params + n_outs),
            out_specs=(PartitionSpec("core"),) * n_outs,
            check_rep=False,
        ),
        keep_unused=True,
    )
    per_core = [[np.asarray(m[name]) for name in in_names[:n_params]] for m in in_maps]
    concat_in = [
        np.concatenate([per_core[c][i] for c in range(N_CORES)], axis=0)
        for i in range(n_params)
    ]
    concat_zeros = [
        np.zeros((N_CORES * z.shape[0], *z.shape[1:]), z.dtype) for z in zero_outs
    ]
    sh = NamedSharding(mesh, PartitionSpec("core"))
    args_dev = [jax.device_put(a, sh) for a in concat_in] + [
        jax.device_put(a, sh) for a in concat_zeros
    ]

    def run():
        import jax as _jax

        outs = sharded(*args_dev)
        _jax.block_until_ready(outs)
        return [
            {
                name: np.asarray(outs[i]).reshape(N_CORES, *out_avals[i].shape)[c]
                for i, name in enumerate(out_names)
            }
            for c in range(N_CORES)
        ]

    def run_nofetch():
        import jax as _jax

        outs = sharded(*args_dev)
        _jax.block_until_ready(outs)

    run.nofetch = run_nofetch
    return run


def kernel(hidden_states, attention_mask, Wq, bq, Wk, bk, Wv, bv):
    in_maps = host_prep(hidden_states, attention_mask, Wq, bq, Wk, bk, Wv, bv)
    nc = build_bass()
    res = bass_utils.run_bass_kernel_spmd(nc, in_maps, core_ids=list(range(N_CORES)))
    return gather(res.results, bv)
